# revision 24
# baseline (speedup 1.0000x reference)
"""Trainium2 Bass kernel for a 2-layer GAT + global mean pool + linear head.

Math (matches PyG GATConv, eval mode, single head, add_self_loops=True):
  h   = x @ W
  e_k = lrelu(ss[src_k] + sd[dst_k]),  ss = h@a_src, sd = h@a_dst
  alpha = softmax over incoming edges of each dst (self-loop included)
  out[d] = sum_k alpha_k h[src_k] + b
Two GAT layers (512->128, 128->64) with ReLU, then per-graph mean pool
over `batch` and a final [64,2] linear.

Strategy (8 NeuronCores, full inputs in / full output out):
  * Destination nodes sharded across cores (2500/core), sources arbitrary.
  * Fused projection: W' = [W | W@a_src | W@a_dst] so one bf16 matmul
    yields h, ss and sd per node.  Each core computes its shard's table
    rows [h | ss | pad] (bf16, 512B for layer 1 / 256B for layer 2) and
    AllGathers the table into every core's HBM.
  * Edges grouped per destination into fixed "slots" (padded with a
    sentinel table row that contributes ~0 to the softmax), destination-
    per-partition.  Slot rows are fetched with SWDGE dma_gather, 1024
    indices (8 slot columns x 128 dests) per instruction.
  * Attention: DVE lrelu (3 small ops) + ACT Exp with accum_out =
    softmax denominator.  Aggregation: bf16 DVE broadcast-multiply +
    strided tensor_reduce.  Dense matmuls/transposes/pooling on PE.
  * Per-graph pooling one-hots (with 1/count folded in) are host-built;
    partial pooled features are AllReduced, final linear on every core.

All graph-structure preprocessing (degree sort, slot layout, index
remapping, SWDGE index wrapping) is host-side numpy on the kernel
inputs; the device only sees dense arrays.
"""

import math
import numpy as np

import concourse.bass as bass
import concourse.bacc as bacc
import concourse.mybir as mybir
from concourse.tile import TileContext
from concourse.masks import make_identity
from concourse.bass_utils import run_bass_kernel_spmd

F32 = mybir.dt.float32
BF16 = mybir.dt.bfloat16
I16 = mybir.dt.int16
AF = mybir.ActivationFunctionType
ALU = mybir.AluOpType

NEG_SLOPE = 0.2
SENT_SS = -60.0  # sentinel row score: exp(lrelu(-60+sd)) ~ e^-12 -> harmless


def full_cfg():
    return dict(N=20000, IND=512, HID=128, HID2=64, OUT=2, G=16, NCORES=8,
                LCAP=8, R1=256, R2=128)


# ----------------------------------------------------------------------------
# Host-side preprocessing
# ----------------------------------------------------------------------------

def preprocess(x, edge_index, batch, W1, a1_src, a1_dst, b1,
               W2, a2_src, a2_dst, b2, Wl, bl, cfg):
    import ml_dtypes
    N, IND, HID, HID2, OUT, G, NC, LCAP, R1, R2 = (
        cfg[k] for k in ("N", "IND", "HID", "HID2", "OUT", "G", "NCORES",
                         "LCAP", "R1", "R2"))
    PC = math.ceil(N / NC)            # real dests per core
    PB = math.ceil(PC / 128)          # dest blocks per core
    PCP = PB * 128                    # padded dests per core
    TR = NC * PCP + 1                 # table rows (+1 sentinel)
    SENT = TR - 1
    H1W = HID + 2      # fused matmul width: [h | ss | sd]
    H2W = HID2 + 2

    x = np.asarray(x, np.float32)
    batch = np.asarray(batch, np.int64)
    src = np.asarray(edge_index[0], np.int64)
    dst = np.asarray(edge_index[1], np.int64)
    # self loops
    loop = np.arange(N, dtype=np.int64)
    src = np.concatenate([src, loop])
    dst = np.concatenate([dst, loop])

    counts = np.bincount(batch, minlength=G).astype(np.float64)

    # per-core degree-sorted permutations and global row ids
    row_of = np.empty(N, np.int64)       # global node -> table row
    orders = []
    degs_sorted = np.zeros((NC, PCP), np.int64)
    for k in range(NC):
        lo, hi = k * PC, min((k + 1) * PC, N)
        nk = hi - lo
        mask = (dst >= lo) & (dst < hi)
        deg = np.bincount(dst[mask] - lo, minlength=nk)
        order = np.argsort(-deg, kind="stable")        # local rank -> local id
        inv = np.empty(nk, np.int64)
        inv[order] = np.arange(nk)
        row_of[lo:hi] = k * PCP + inv
        orders.append(order)
        degs_sorted[k, :nk] = deg[order]

    # global per-block slot counts (identical program on every core)
    Ls = []
    for b in range(PB):
        Lb = int(degs_sorted[:, b * 128:(b + 1) * 128].max())
        Ls.append(max(Lb, 1))
    S = int(np.sum(Ls))
    offs = np.concatenate([[0], np.cumsum(Ls)]).astype(np.int64)
    # sub-block split (shared host/device)
    subs = []                            # (b, s0, Lc, c0)
    for b in range(PB):
        for s0 in range(0, Ls[b], LCAP):
            Lc = min(LCAP, Ls[b] - s0)
            subs.append((b, s0, Lc, int(offs[b]) + s0))

    # fused replicated weights: W' = [W | W@a_src | W@a_dst]  (bf16)
    KB = IND // 128
    W1f = W1.astype(np.float64)
    W1p = np.concatenate([W1f, (W1f @ a1_src.astype(np.float64))[:, None],
                          (W1f @ a1_dst.astype(np.float64))[:, None]], axis=1)
    W1u = np.ascontiguousarray(
        W1p.reshape(KB, 128, H1W)).astype(ml_dtypes.bfloat16)
    W2f = W2.astype(np.float64)
    W2p = np.concatenate([W2f, (W2f @ a2_src.astype(np.float64))[:, None],
                          (W2f @ a2_dst.astype(np.float64))[:, None]], axis=1)
    W2u = np.ascontiguousarray(W2p).astype(ml_dtypes.bfloat16)
    b1r = np.tile(np.asarray(b1, np.float32)[None, :], (128, 1))
    b2r = np.tile(np.asarray(b2, np.float32)[None, :], (128, 1))
    WlBl = np.concatenate([np.asarray(Wl, np.float32),
                           np.asarray(bl, np.float32)[None, :]], axis=0)
    sent1 = np.zeros((1, R1), ml_dtypes.bfloat16)
    sent1[0, HID] = SENT_SS
    sent2 = np.zeros((1, R2), ml_dtypes.bfloat16)
    sent2[0, HID2] = SENT_SS

    in_maps = []
    for k in range(NC):
        lo, hi = k * PC, min((k + 1) * PC, N)
        nk = hi - lo
        order = orders[k]

        # xT: [KB, 128, PCP] bf16 (feature-major columns in local-rank order)
        xs = np.zeros((PCP, IND), np.float32)
        xs[:nk] = x[lo:hi][order]
        xT = np.ascontiguousarray(
            xs.T.reshape(KB, 128, PCP)).astype(ml_dtypes.bfloat16)

        # slot indices [128, S] -> table rows, sentinel padded
        sidx = np.full((128, S), SENT, np.int64)
        mask = (dst >= lo) & (dst < hi)
        es, ed = src[mask], dst[mask] - lo
        o = np.argsort(ed, kind="stable")
        es, ed = es[o], ed[o]
        deg = np.bincount(ed, minlength=nk)
        start = np.concatenate([[0], np.cumsum(deg)[:-1]])
        j = np.arange(len(ed)) - start[ed]            # slot within dest
        inv = np.empty(nk, np.int64)
        inv[order] = np.arange(nk)
        r = inv[ed]                                   # dest rank
        bb, pp = r // 128, r % 128
        col = offs[bb] + j
        sidx[pp, col] = row_of[es]

        # SWDGE wrapped indices: per sub-block, j = l*128 + p ->
        # wrapped[j%16, base + j//16]; 16-row wrap replicated to 128.
        sw = np.zeros((16, S * 8), np.int16)
        for (b, s0, Lc, c0) in subs:
            jl = sidx[:, c0:c0 + Lc]                  # [128, Lc]
            flat = jl.T.ravel().astype(np.int16)      # j = l*128 + p
            sw[:, c0 * 8:(c0 + Lc) * 8] = flat.reshape(-1, 16).T
        sidx_w = np.tile(sw, (8, 1))                  # [128, S*8]

        # pooling one-hot with 1/count folded, zero rows for pad dests
        P = np.zeros((128, PB * G), np.float32)
        bg = batch[lo:hi][order]                      # graph id per rank
        rr = np.arange(nk)
        P[rr % 128, (rr // 128) * G + bg] = 1.0 / np.maximum(counts[bg], 1.0)

        in_maps.append(dict(
            xT=xT, W1u=W1u, W2u=W2u, b1r=b1r, b2r=b2r,
            WlBl=WlBl.astype(np.float32),
            Pp=P, sidxw=sidx_w, sent1=sent1, sent2=sent2,
        ))

    meta = dict(PC=PC, PB=PB, PCP=PCP, TR=TR, KB=KB, S=S,
                H1W=H1W, H2W=H2W, Ls=Ls, offs=offs, subs=subs)
    return in_maps, meta


# ----------------------------------------------------------------------------
# Device program
# ----------------------------------------------------------------------------

def build_program(cfg, meta, reps=1, debug_outs=False, phases=3):
    N, IND, HID, HID2, OUT, G, NC, LCAP, R1, R2 = (
        cfg[k] for k in ("N", "IND", "HID", "HID2", "OUT", "G", "NCORES",
                         "LCAP", "R1", "R2"))
    PB, PCP, TR, KB, S, H1W, H2W = (meta[k] for k in
                                    ("PB", "PCP", "TR", "KB", "S", "H1W",
                                     "H2W"))
    Ls, offs, subs = meta["Ls"], meta["offs"], meta["subs"]

    nc = bacc.Bacc("TRN2", target_bir_lowering=False, debug=False,
                   num_devices=NC, num_swdge_queues=4)

    xT_d = nc.declare_dram_parameter("xT", [KB, 128, PCP], BF16, False)
    W1_d = nc.declare_dram_parameter("W1u", [KB, 128, H1W], BF16, False)
    W2_d = nc.declare_dram_parameter("W2u", [HID, H2W], BF16, False)
    b1r_d = nc.declare_dram_parameter("b1r", [128, HID], F32, False)
    b2r_d = nc.declare_dram_parameter("b2r", [128, HID2], F32, False)
    Wl_d = nc.declare_dram_parameter("WlBl", [HID2 + 1, OUT], F32, False)
    Pp_d = nc.declare_dram_parameter("Pp", [128, PB * G], F32, False)
    sidx_d = nc.declare_dram_parameter("sidxw", [128, S * 8], I16, False)
    sent1_d = nc.declare_dram_parameter("sent1", [1, R1], BF16, False)
    sent2_d = nc.declare_dram_parameter("sent2", [1, R2], BF16, False)
    out_d = nc.declare_dram_parameter("out", [G, OUT], F32, True)
    if debug_outs:
        dbg_t1 = nc.declare_dram_parameter("dbg_t1", [TR, R1], BF16, True)
        dbg_g = nc.declare_dram_parameter("dbg_g", [128, Ls[0] * R1], BF16,
                                          True)
        dbg_r1 = nc.declare_dram_parameter("dbg_r1", [PB * 128, HID], F32,
                                           True)

    shared = dict(addr_space="Shared") if NC > 4 else {}
    T1shard = nc.dram_tensor("T1shard", [PCP, R1], BF16)
    T1full = nc.dram_tensor("T1full", [TR, R1], BF16, **shared)
    T2shard = nc.dram_tensor("T2shard", [PCP, R2], BF16)
    T2full = nc.dram_tensor("T2full", [TR, R2], BF16, **shared)
    poolin = nc.dram_tensor("poolin", [G, HID2], F32)
    poolout = nc.dram_tensor("poolout", [G, HID2], F32, **shared)

    groups = [list(range(NC))]

    with TileContext(nc) as tc:
        with (
            tc.tile_pool(name="const", bufs=1) as cp,
            tc.tile_pool(name="work", bufs=3) as wp,
            tc.tile_pool(name="wtp", bufs=1) as wtp,
            tc.tile_pool(name="xpool", bufs=3) as xp,
            tc.tile_pool(name="psA", bufs=2, space="PSUM") as psA,
            tc.tile_pool(name="psB", bufs=2, space="PSUM") as psB,
            tc.tile_pool(name="psP", bufs=1, space="PSUM") as psP,
        ):
            for _rep in range(reps):
                # ---------------- constants to SBUF ----------------
                W1_sb = cp.tile([128, KB * H1W], BF16, tag="w1")
                W1v = W1_sb[:].rearrange("p (k h) -> p k h", h=H1W)
                nc.sync.dma_start(
                    out=W1v, in_=W1_d[:].rearrange("k p h -> p k h"))
                W2_sb = cp.tile([HID, H2W], BF16, tag="w2")
                nc.sync.dma_start(out=W2_sb[:], in_=W2_d[:])
                b1r_sb = cp.tile([128, HID], F32, tag="b1r")
                nc.sync.dma_start(out=b1r_sb[:], in_=b1r_d[:])
                b2r_sb = cp.tile([128, HID2], F32, tag="b2r")
                nc.sync.dma_start(out=b2r_sb[:], in_=b2r_d[:])
                Wl_sb = cp.tile([HID2 + 1, OUT], F32, tag="wl")
                nc.sync.dma_start(out=Wl_sb[:], in_=Wl_d[:])
                P_sb = cp.tile([128, PB * G], F32, tag="pp")
                nc.sync.dma_start(out=P_sb[:], in_=Pp_d[:])
                sidx_sb = cp.tile([128, S * 8], I16, tag="sidx")
                nc.sync.dma_start(out=sidx_sb[:], in_=sidx_d[:])
                ident = cp.tile([128, 128], F32, tag="id")
                make_identity(nc, ident[:])

                T1sb = cp.tile([128, PB * R1], BF16, tag="t1")
                nc.vector.memset(T1sb[:], 0.0)
                T2sb = cp.tile([128, PB * R2], BF16, tag="t2")
                nc.vector.memset(T2sb[:], 0.0)
                sd1 = cp.tile([128, PB], F32, tag="sd1")
                sd2 = cp.tile([128, PB], F32, tag="sd2")

                # ---------------- phase A: h1 / scores / T1 ----------------
                for c in range(PB):
                    xc = xp.tile([128, KB * 128], BF16, tag="xc")
                    xcv = xc[:].rearrange("p (k n) -> p k n", n=128)
                    nc.sync.dma_start(
                        out=xcv,
                        in_=xT_d[:, :, c * 128:(c + 1) * 128]
                        .rearrange("k p n -> p k n"))
                    ph = psA.tile([128, H1W], F32, tag="ph")
                    for kb in range(KB):
                        nc.tensor.matmul(
                            ph[:],
                            lhsT=xc[:, kb * 128:(kb + 1) * 128],
                            rhs=W1_sb[:, kb * H1W:(kb + 1) * H1W],
                            start=(kb == 0), stop=(kb == KB - 1),
                        )
                    nc.vector.tensor_copy(
                        T1sb[:, c * R1:c * R1 + HID + 1], ph[:, 0:HID + 1])
                    nc.vector.tensor_copy(
                        sd1[:, c:c + 1], ph[:, HID + 1:HID + 2])
                    nc.sync.dma_start(
                        out=T1shard[c * 128:(c + 1) * 128, :],
                        in_=T1sb[:, c * R1:(c + 1) * R1])
                nc.sync.dma_start(out=T1full[TR - 1:TR, :], in_=sent1_d[:])
                nc.gpsimd.collective_compute(
                    "AllGather", ALU.bypass, replica_groups=groups,
                    ins=[T1shard[:]], outs=[T1full[0:TR - 1, :]])

                # ---------------- phase B: GAT layer 1 ----------------
                if phases < 2:
                    out_sb = wp.tile([G, OUT], F32, tag="outsb")
                    nc.vector.memset(out_sb[:], 0.0)
                    nc.sync.dma_start(out=out_d[:], in_=out_sb[:])
                    continue
                qi = 0
                gpB = tc.tile_pool(name="gathB", bufs=4)
                gp = gpB.__enter__()
                for b in range(PB):
                    L = Ls[b]
                    c0b = int(offs[b])
                    Gt = gp.tile([128, Ls[0] * R1], BF16, tag="g1")
                    Gv = Gt[:, 0:L * R1].rearrange("p (l w) -> p l w", w=R1)
                    for s0 in range(0, L, LCAP):
                        Lc = min(LCAP, L - s0)
                        c0 = c0b + s0
                        nc.gpsimd.dma_gather(
                            out_ap=Gt[:, s0 * R1:(s0 + Lc) * R1].rearrange(
                                "p (l w) -> p l w", w=R1),
                            in_ap=T1full[:],
                            idxs_ap=sidx_sb[:, c0 * 8:(c0 + Lc) * 8],
                            num_idxs=128 * Lc, num_idxs_reg=128 * Lc,
                            elem_size=R1, queue_num=qi % 4)
                        qi += 1
                    if debug_outs and b == 0:
                        nc.sync.dma_start(
                            out=dbg_g[:, 0:L * R1], in_=Gt[:, 0:L * R1])
                    t_t = wp.tile([128, L], F32, tag="tpre")
                    nc.vector.tensor_scalar(
                        out=t_t[:], in0=Gv[:, :, HID],
                        scalar1=sd1[:, b:b + 1], scalar2=None, op0=ALU.add)
                    u_t = wp.tile([128, L], F32, tag="upre")
                    nc.vector.tensor_scalar(
                        out=u_t[:], in0=t_t[:], scalar1=NEG_SLOPE,
                        scalar2=None, op0=ALU.mult)
                    wl_t = wp.tile([128, L], F32, tag="wl1")
                    nc.vector.tensor_tensor(
                        out=wl_t[:], in0=t_t[:], in1=u_t[:], op=ALU.max)
                    wex = wp.tile([128, L], F32, tag="we1")
                    den = wp.tile([128, 1], F32, tag="den")
                    nc.scalar.activation(
                        wex[:], wl_t[:], AF.Exp, accum_out=den[:])
                    wexb = wp.tile([128, L], BF16, tag="web")
                    nc.vector.tensor_copy(wexb[:], wex[:])
                    wt = wtp.tile([128, Ls[0] * HID], BF16, tag="wt")
                    wtv = wt[:, 0:L * HID]
                    nc.vector.tensor_tensor(
                        out=wtv, in0=Gv[:, :, 0:HID],
                        in1=wexb[:, :, None].to_broadcast([128, L, HID]),
                        op=ALU.mult)
                    o_t = wp.tile([128, HID], F32, tag="o1")
                    nc.vector.tensor_reduce(
                        out=o_t[:],
                        in_=wtv.rearrange("p (l f) -> p f l", f=HID),
                        axis=mybir.AxisListType.X, op=ALU.add)
                    # ---- block epilogue: normalize, relu, layer-2 matmul
                    rec = wp.tile([128, 1], F32, tag="rec")
                    nc.vector.reciprocal(rec[:], den[:])
                    ob = wp.tile([128, HID], F32, tag="ob")
                    nc.vector.scalar_tensor_tensor(
                        out=ob[:], in0=o_t[:], scalar=rec[:], in1=b1r_sb[:],
                        op0=ALU.mult, op1=ALU.add)
                    r1 = wp.tile([128, HID], F32, tag="r1")
                    nc.scalar.activation(r1[:], ob[:], AF.Relu)
                    if debug_outs:
                        nc.sync.dma_start(
                            out=dbg_r1[b * 128:(b + 1) * 128, :], in_=r1[:])
                        if b == 0:
                            nc.sync.dma_start(out=dbg_t1[:], in_=T1full[:])
                    pT = psB.tile([128, HID], F32, tag="tr")
                    nc.tensor.transpose(pT[:], r1[:], identity=ident[:])
                    r1T = wp.tile([128, HID], BF16, tag="r1T")
                    nc.vector.tensor_copy(r1T[:], pT[:])
                    ph2 = psB.tile([128, H2W], F32, tag="tr2")
                    nc.tensor.matmul(ph2[:], lhsT=r1T[:], rhs=W2_sb[:],
                                     start=True, stop=True)
                    nc.vector.tensor_copy(
                        T2sb[:, b * R2:b * R2 + HID2 + 1],
                        ph2[:, 0:HID2 + 1])
                    nc.vector.tensor_copy(
                        sd2[:, b:b + 1], ph2[:, HID2 + 1:HID2 + 2])
                    nc.sync.dma_start(
                        out=T2shard[b * 128:(b + 1) * 128, :],
                        in_=T2sb[:, b * R2:(b + 1) * R2])

                gpB.__exit__(None, None, None)
                nc.sync.dma_start(out=T2full[TR - 1:TR, :], in_=sent2_d[:])
                nc.gpsimd.collective_compute(
                    "AllGather", ALU.bypass, replica_groups=groups,
                    ins=[T2shard[:]], outs=[T2full[0:TR - 1, :]])

                # ------------- phase C: GAT layer 2 + pooling -------------
                if phases < 3:
                    out_sb = wp.tile([G, OUT], F32, tag="outsb")
                    nc.vector.memset(out_sb[:], 0.0)
                    nc.sync.dma_start(out=out_d[:], in_=out_sb[:])
                    continue
                pool_ps = psP.tile([G, HID2], F32, tag="pool")
                qi = 0
                gpC = tc.tile_pool(name="gathC", bufs=4)
                gp = gpC.__enter__()
                for b in range(PB):
                    L = Ls[b]
                    c0b = int(offs[b])
                    Gt = gp.tile([128, Ls[0] * R2], BF16, tag="g2")
                    Gv = Gt[:, 0:L * R2].rearrange("p (l w) -> p l w", w=R2)
                    for s0 in range(0, L, LCAP):
                        Lc = min(LCAP, L - s0)
                        c0 = c0b + s0
                        nc.gpsimd.dma_gather(
                            out_ap=Gt[:, s0 * R2:(s0 + Lc) * R2].rearrange(
                                "p (l w) -> p l w", w=R2),
                            in_ap=T2full[:],
                            idxs_ap=sidx_sb[:, c0 * 8:(c0 + Lc) * 8],
                            num_idxs=128 * Lc, num_idxs_reg=128 * Lc,
                            elem_size=R2, queue_num=qi % 4)
                        qi += 1
                    t_t = wp.tile([128, L], F32, tag="tpre")
                    nc.vector.tensor_scalar(
                        out=t_t[:], in0=Gv[:, :, HID2],
                        scalar1=sd2[:, b:b + 1], scalar2=None, op0=ALU.add)
                    u_t = wp.tile([128, L], F32, tag="upre")
                    nc.vector.tensor_scalar(
                        out=u_t[:], in0=t_t[:], scalar1=NEG_SLOPE,
                        scalar2=None, op0=ALU.mult)
                    wl_t = wp.tile([128, L], F32, tag="wl1")
                    nc.vector.tensor_tensor(
                        out=wl_t[:], in0=t_t[:], in1=u_t[:], op=ALU.max)
                    wex = wp.tile([128, L], F32, tag="we1")
                    den = wp.tile([128, 1], F32, tag="den")
                    nc.scalar.activation(
                        wex[:], wl_t[:], AF.Exp, accum_out=den[:])
                    wexb = wp.tile([128, L], BF16, tag="web")
                    nc.vector.tensor_copy(wexb[:], wex[:])
                    wt = wtp.tile([128, Ls[0] * HID2], BF16, tag="wt2")
                    wtv = wt[:, 0:L * HID2]
                    nc.vector.tensor_tensor(
                        out=wtv, in0=Gv[:, :, 0:HID2],
                        in1=wexb[:, :, None].to_broadcast([128, L, HID2]),
                        op=ALU.mult)
                    o_t = wp.tile([128, HID2], F32, tag="o2")
                    nc.vector.tensor_reduce(
                        out=o_t[:],
                        in_=wtv.rearrange("p (l f) -> p f l", f=HID2),
                        axis=mybir.AxisListType.X, op=ALU.add)
                    rec = wp.tile([128, 1], F32, tag="rec")
                    nc.vector.reciprocal(rec[:], den[:])
                    ob = wp.tile([128, HID2], F32, tag="ob2")
                    nc.vector.scalar_tensor_tensor(
                        out=ob[:], in0=o_t[:], scalar=rec[:], in1=b2r_sb[:],
                        op0=ALU.mult, op1=ALU.add)
                    r2 = wp.tile([128, HID2], F32, tag="r2")
                    nc.scalar.activation(r2[:], ob[:], AF.Relu)
                    nc.tensor.matmul(
                        pool_ps[:], lhsT=P_sb[:, b * G:(b + 1) * G],
                        rhs=r2[:], start=(b == 0), stop=(b == PB - 1))

                if phases == 4:
                    out_sb = wp.tile([G, OUT], F32, tag="outsb")
                    nc.vector.memset(out_sb[:], 0.0)
                    nc.sync.dma_start(out=out_d[:], in_=out_sb[:])
                    pooled = wp.tile([G, HID2], F32, tag="pool")
                    nc.vector.tensor_copy(pooled[:], pool_ps[:])
                    continue
                gpC.__exit__(None, None, None)
                pooled = wp.tile([G, HID2], F32, tag="pool")
                nc.vector.tensor_copy(pooled[:], pool_ps[:])
                nc.sync.dma_start(out=poolin[:], in_=pooled[:])
                nc.gpsimd.collective_compute(
                    "AllReduce", ALU.add, replica_groups=groups,
                    ins=[poolin[:]], outs=[poolout[:]])
                pooled_r = wp.tile([G, HID2], F32, tag="poolr")
                nc.sync.dma_start(out=pooled_r[:], in_=poolout[:])
                pTf = psB.tile([HID2, G], F32, tag="tr")
                nc.tensor.transpose(pTf[:], pooled_r[:],
                                    identity=ident[:G, :G])
                fin = wp.tile([HID2 + 1, G], F32, tag="fin")
                nc.vector.tensor_copy(fin[:HID2, :], pTf[:])
                nc.vector.memset(fin[HID2:HID2 + 1, :], 1.0)
                out_ps = psP.tile([G, OUT], F32, tag="tro")
                nc.tensor.matmul(out_ps[:], lhsT=fin[:], rhs=Wl_sb[:],
                                 start=True, stop=True)
                out_sb = wp.tile([G, OUT], F32, tag="outsb")
                nc.vector.tensor_copy(out_sb[:], out_ps[:])
                nc.sync.dma_start(out=out_d[:], in_=out_sb[:])

    nc.compile()
    return nc


# ----------------------------------------------------------------------------
# Entry point
# ----------------------------------------------------------------------------

LAST_RESULTS = None


def kernel(**inputs):
    global LAST_RESULTS
    cfg = full_cfg()
    in_maps, meta = preprocess(cfg=cfg, **inputs)
    nc = build_program(cfg, meta)
    res = run_bass_kernel_spmd(nc, in_maps,
                               core_ids=list(range(cfg["NCORES"])))
    LAST_RESULTS = res
    return np.asarray(res.results[0]["out"], np.float32)


# revision 27
# speedup vs baseline: 1.2778x; 1.2778x over previous
"""Trainium2 Bass kernel for a 2-layer GAT + global mean pool + linear head.

Math (matches PyG GATConv, eval mode, single head, add_self_loops=True):
  h   = x @ W
  e_k = lrelu(ss[src_k] + sd[dst_k]),  ss = h@a_src, sd = h@a_dst
  alpha = softmax over incoming edges of each dst (self-loop included)
  out[d] = sum_k alpha_k h[src_k] + b
Two GAT layers (512->128, 128->64) with ReLU, then per-graph mean pool
over `batch` and a final [64,2] linear.

Strategy (8 NeuronCores, full inputs in / full output out):
  * Destination nodes sharded across cores (2500/core), sources arbitrary.
  * Fused projection: W' = [W | W@a_src | W@a_dst] so one bf16 matmul
    yields h, ss and sd per node.  Each core computes its shard's table
    rows [h | ss | pad] (bf16, 512B for layer 1 / 256B for layer 2) and
    AllGathers the table into every core's HBM.
  * Edges grouped per destination into fixed "slots" (padded with a
    sentinel table row that contributes ~0 to the softmax), destination-
    per-partition.  Slot rows are fetched with SWDGE dma_gather, 1024
    indices (8 slot columns x 128 dests) per instruction.
  * Attention: DVE lrelu (3 small ops) + ACT Exp with accum_out =
    softmax denominator.  Aggregation: bf16 DVE broadcast-multiply +
    strided tensor_reduce.  Dense matmuls/transposes/pooling on PE.
  * Per-graph pooling one-hots (with 1/count folded in) are host-built;
    partial pooled features are AllReduced, final linear on every core.

All graph-structure preprocessing (degree sort, slot layout, index
remapping, SWDGE index wrapping) is host-side numpy on the kernel
inputs; the device only sees dense arrays.
"""

import math
import numpy as np

import concourse.bass as bass
import concourse.bacc as bacc
import concourse.mybir as mybir
from concourse.tile import TileContext
from concourse.masks import make_identity
from concourse.bass_utils import run_bass_kernel_spmd

F32 = mybir.dt.float32
BF16 = mybir.dt.bfloat16
I16 = mybir.dt.int16
AF = mybir.ActivationFunctionType
ALU = mybir.AluOpType

NEG_SLOPE = 0.2
SENT_SS = -60.0  # sentinel row score: exp(lrelu(-60+sd)) ~ e^-12 -> harmless


def full_cfg():
    return dict(N=20000, IND=512, HID=128, HID2=64, OUT=2, G=16, NCORES=8,
                LCAP=8, R1=256, R2=128)


# ----------------------------------------------------------------------------
# Host-side preprocessing
# ----------------------------------------------------------------------------

def preprocess(x, edge_index, batch, W1, a1_src, a1_dst, b1,
               W2, a2_src, a2_dst, b2, Wl, bl, cfg):
    import ml_dtypes
    N, IND, HID, HID2, OUT, G, NC, LCAP, R1, R2 = (
        cfg[k] for k in ("N", "IND", "HID", "HID2", "OUT", "G", "NCORES",
                         "LCAP", "R1", "R2"))
    PC = math.ceil(N / NC)            # real dests per core
    PB = math.ceil(PC / 128)          # dest blocks per core
    PCP = PB * 128                    # padded dests per core
    TR = NC * PCP + 1                 # table rows (+1 sentinel)
    SENT = TR - 1
    H1W = HID + 2      # fused matmul width: [h | ss | sd]
    H2W = HID2 + 2

    x = np.asarray(x, np.float32)
    batch = np.asarray(batch, np.int64)
    src = np.asarray(edge_index[0], np.int64)
    dst = np.asarray(edge_index[1], np.int64)
    # self loops
    loop = np.arange(N, dtype=np.int64)
    src = np.concatenate([src, loop])
    dst = np.concatenate([dst, loop])

    counts = np.bincount(batch, minlength=G).astype(np.float64)

    # per-core degree-sorted permutations and global row ids
    row_of = np.empty(N, np.int64)       # global node -> table row
    orders = []
    degs_sorted = np.zeros((NC, PCP), np.int64)
    for k in range(NC):
        lo, hi = k * PC, min((k + 1) * PC, N)
        nk = hi - lo
        mask = (dst >= lo) & (dst < hi)
        deg = np.bincount(dst[mask] - lo, minlength=nk)
        order = np.argsort(-deg, kind="stable")        # local rank -> local id
        inv = np.empty(nk, np.int64)
        inv[order] = np.arange(nk)
        row_of[lo:hi] = k * PCP + inv
        orders.append(order)
        degs_sorted[k, :nk] = deg[order]

    # global per-block slot counts (identical program on every core)
    Ls = []
    for b in range(PB):
        Lb = int(degs_sorted[:, b * 128:(b + 1) * 128].max())
        Ls.append(max(Lb, 1))
    S = int(np.sum(Ls))
    offs = np.concatenate([[0], np.cumsum(Ls)]).astype(np.int64)
    # sub-block split (shared host/device)
    subs = []                            # (b, s0, Lc, c0)
    for b in range(PB):
        for s0 in range(0, Ls[b], LCAP):
            Lc = min(LCAP, Ls[b] - s0)
            subs.append((b, s0, Lc, int(offs[b]) + s0))

    # fused replicated weights: W' = [W | W@a_src | W@a_dst]  (bf16)
    KB = IND // 128
    W1f = W1.astype(np.float64)
    W1p = np.concatenate([W1f, (W1f @ a1_src.astype(np.float64))[:, None],
                          (W1f @ a1_dst.astype(np.float64))[:, None]], axis=1)
    W1u = np.ascontiguousarray(
        W1p.reshape(KB, 128, H1W)).astype(ml_dtypes.bfloat16)
    W2f = W2.astype(np.float64)
    W2p = np.concatenate([W2f, (W2f @ a2_src.astype(np.float64))[:, None],
                          (W2f @ a2_dst.astype(np.float64))[:, None]], axis=1)
    W2u = np.ascontiguousarray(W2p).astype(ml_dtypes.bfloat16)
    b1r = np.tile(np.asarray(b1, np.float32)[None, :], (128, 1))
    b2r = np.tile(np.asarray(b2, np.float32)[None, :], (128, 1))
    WlBl = np.concatenate([np.asarray(Wl, np.float32),
                           np.asarray(bl, np.float32)[None, :]], axis=0)
    sent1 = np.zeros((1, R1), ml_dtypes.bfloat16)
    sent1[0, HID] = SENT_SS
    sent2 = np.zeros((1, R2), ml_dtypes.bfloat16)
    sent2[0, HID2] = SENT_SS

    in_maps = []
    for k in range(NC):
        lo, hi = k * PC, min((k + 1) * PC, N)
        nk = hi - lo
        order = orders[k]

        # xT: [KB, 128, PCP] bf16 (feature-major columns in local-rank order)
        xs = np.zeros((PCP, IND), np.float32)
        xs[:nk] = x[lo:hi][order]
        xT = np.ascontiguousarray(
            xs.T.reshape(KB, 128, PCP)).astype(ml_dtypes.bfloat16)

        # slot indices [128, S] -> table rows, sentinel padded
        sidx = np.full((128, S), SENT, np.int64)
        mask = (dst >= lo) & (dst < hi)
        es, ed = src[mask], dst[mask] - lo
        o = np.argsort(ed, kind="stable")
        es, ed = es[o], ed[o]
        deg = np.bincount(ed, minlength=nk)
        start = np.concatenate([[0], np.cumsum(deg)[:-1]])
        j = np.arange(len(ed)) - start[ed]            # slot within dest
        inv = np.empty(nk, np.int64)
        inv[order] = np.arange(nk)
        r = inv[ed]                                   # dest rank
        bb, pp = r // 128, r % 128
        col = offs[bb] + j
        sidx[pp, col] = row_of[es]

        # SWDGE wrapped indices: per sub-block, j = l*128 + p ->
        # wrapped[j%16, base + j//16]; 16-row wrap replicated to 128.
        sw = np.zeros((16, S * 8), np.int16)
        for (b, s0, Lc, c0) in subs:
            jl = sidx[:, c0:c0 + Lc]                  # [128, Lc]
            flat = jl.T.ravel().astype(np.int16)      # j = l*128 + p
            sw[:, c0 * 8:(c0 + Lc) * 8] = flat.reshape(-1, 16).T
        sidx_w = np.tile(sw, (8, 1))                  # [128, S*8]

        # pooling one-hot with 1/count folded, zero rows for pad dests
        P = np.zeros((128, PB * G), np.float32)
        bg = batch[lo:hi][order]                      # graph id per rank
        rr = np.arange(nk)
        P[rr % 128, (rr // 128) * G + bg] = 1.0 / np.maximum(counts[bg], 1.0)

        in_maps.append(dict(
            xT=xT, W1u=W1u, W2u=W2u, b1r=b1r, b2r=b2r,
            WlBl=WlBl.astype(np.float32),
            Pp=P, sidxw=sidx_w, sent1=sent1, sent2=sent2,
        ))

    meta = dict(PC=PC, PB=PB, PCP=PCP, TR=TR, KB=KB, S=S,
                H1W=H1W, H2W=H2W, Ls=Ls, offs=offs, subs=subs)
    return in_maps, meta


# ----------------------------------------------------------------------------
# Device program
# ----------------------------------------------------------------------------

def build_program(cfg, meta, reps=1, debug_outs=False, phases=3):
    N, IND, HID, HID2, OUT, G, NC, LCAP, R1, R2 = (
        cfg[k] for k in ("N", "IND", "HID", "HID2", "OUT", "G", "NCORES",
                         "LCAP", "R1", "R2"))
    PB, PCP, TR, KB, S, H1W, H2W = (meta[k] for k in
                                    ("PB", "PCP", "TR", "KB", "S", "H1W",
                                     "H2W"))
    Ls, offs, subs = meta["Ls"], meta["offs"], meta["subs"]

    nc = bacc.Bacc("TRN2", target_bir_lowering=False, debug=False,
                   num_devices=NC, num_swdge_queues=4)

    xT_d = nc.declare_dram_parameter("xT", [KB, 128, PCP], BF16, False)
    W1_d = nc.declare_dram_parameter("W1u", [KB, 128, H1W], BF16, False)
    W2_d = nc.declare_dram_parameter("W2u", [HID, H2W], BF16, False)
    b1r_d = nc.declare_dram_parameter("b1r", [128, HID], F32, False)
    b2r_d = nc.declare_dram_parameter("b2r", [128, HID2], F32, False)
    Wl_d = nc.declare_dram_parameter("WlBl", [HID2 + 1, OUT], F32, False)
    Pp_d = nc.declare_dram_parameter("Pp", [128, PB * G], F32, False)
    sidx_d = nc.declare_dram_parameter("sidxw", [128, S * 8], I16, False)
    sent1_d = nc.declare_dram_parameter("sent1", [1, R1], BF16, False)
    sent2_d = nc.declare_dram_parameter("sent2", [1, R2], BF16, False)
    out_d = nc.declare_dram_parameter("out", [G, OUT], F32, True)
    if debug_outs:
        dbg_t1 = nc.declare_dram_parameter("dbg_t1", [TR, R1], BF16, True)
        dbg_g = nc.declare_dram_parameter("dbg_g", [128, Ls[0] * R1], BF16,
                                          True)
        dbg_r1 = nc.declare_dram_parameter("dbg_r1", [PB * 128, HID], F32,
                                           True)

    shared = dict(addr_space="Shared") if NC > 4 else {}
    T1shard = nc.dram_tensor("T1shard", [PCP, R1], BF16)
    T1full = nc.dram_tensor("T1full", [TR, R1], BF16, **shared)
    T2shard = nc.dram_tensor("T2shard", [PCP, R2], BF16)
    T2full = nc.dram_tensor("T2full", [TR, R2], BF16, **shared)
    poolin = nc.dram_tensor("poolin", [G, HID2], F32)
    poolout = nc.dram_tensor("poolout", [G, HID2], F32, **shared)

    groups = [list(range(NC))]

    with TileContext(nc) as tc:
        with (
            tc.tile_pool(name="const", bufs=1) as cp,
            tc.tile_pool(name="work", bufs=3) as wp,
            tc.tile_pool(name="wtp", bufs=1) as wtp,
            tc.tile_pool(name="xpool", bufs=3) as xp,
            tc.tile_pool(name="psA", bufs=2, space="PSUM") as psA,
            tc.tile_pool(name="psB", bufs=2, space="PSUM") as psB,
            tc.tile_pool(name="psP", bufs=1, space="PSUM") as psP,
        ):
            for _rep in range(reps):
                # ---------------- constants to SBUF ----------------
                W1_sb = cp.tile([128, KB * H1W], BF16, tag="w1")
                W1v = W1_sb[:].rearrange("p (k h) -> p k h", h=H1W)
                nc.sync.dma_start(
                    out=W1v, in_=W1_d[:].rearrange("k p h -> p k h"))
                W2_sb = cp.tile([HID, H2W], BF16, tag="w2")
                nc.sync.dma_start(out=W2_sb[:], in_=W2_d[:])
                b1r_sb = cp.tile([128, HID], F32, tag="b1r")
                nc.sync.dma_start(out=b1r_sb[:], in_=b1r_d[:])
                b2r_sb = cp.tile([128, HID2], F32, tag="b2r")
                nc.sync.dma_start(out=b2r_sb[:], in_=b2r_d[:])
                Wl_sb = cp.tile([HID2 + 1, OUT], F32, tag="wl")
                nc.sync.dma_start(out=Wl_sb[:], in_=Wl_d[:])
                P_sb = cp.tile([128, PB * G], F32, tag="pp")
                nc.sync.dma_start(out=P_sb[:], in_=Pp_d[:])
                sidx_sb = cp.tile([128, S * 8], I16, tag="sidx")
                nc.sync.dma_start(out=sidx_sb[:], in_=sidx_d[:])
                ident = cp.tile([128, 128], F32, tag="id")
                make_identity(nc, ident[:])

                T1sb = cp.tile([128, PB * R1], BF16, tag="t1")
                nc.vector.memset(T1sb[:], 0.0)
                T2sb = cp.tile([128, PB * R2], BF16, tag="t2")
                nc.vector.memset(T2sb[:], 0.0)
                sd1 = cp.tile([128, PB], F32, tag="sd1")
                sd2 = cp.tile([128, PB], F32, tag="sd2")

                # ---------------- phase A: h1 / scores / T1 ----------------
                for c in range(PB):
                    xc = xp.tile([128, KB * 128], BF16, tag="xc")
                    xcv = xc[:].rearrange("p (k n) -> p k n", n=128)
                    nc.sync.dma_start(
                        out=xcv,
                        in_=xT_d[:, :, c * 128:(c + 1) * 128]
                        .rearrange("k p n -> p k n"))
                    ph = psA.tile([128, H1W], F32, tag="ph")
                    for kb in range(KB):
                        nc.tensor.matmul(
                            ph[:],
                            lhsT=xc[:, kb * 128:(kb + 1) * 128],
                            rhs=W1_sb[:, kb * H1W:(kb + 1) * H1W],
                            start=(kb == 0), stop=(kb == KB - 1),
                        )
                    nc.vector.tensor_copy(
                        T1sb[:, c * R1:c * R1 + HID + 1], ph[:, 0:HID + 1])
                    nc.vector.tensor_copy(
                        sd1[:, c:c + 1], ph[:, HID + 1:HID + 2])
                    nc.sync.dma_start(
                        out=T1shard[c * 128:(c + 1) * 128, :],
                        in_=T1sb[:, c * R1:(c + 1) * R1])
                nc.sync.dma_start(out=T1full[TR - 1:TR, :], in_=sent1_d[:])
                nc.gpsimd.collective_compute(
                    "AllGather", ALU.bypass, replica_groups=groups,
                    ins=[T1shard[:]], outs=[T1full[0:TR - 1, :]])

                # ---------------- phase B: GAT layer 1 ----------------
                if phases < 2:
                    out_sb = wp.tile([G, OUT], F32, tag="outsb")
                    nc.vector.memset(out_sb[:], 0.0)
                    nc.sync.dma_start(out=out_d[:], in_=out_sb[:])
                    continue
                qi = 0
                gpB = tc.tile_pool(name="gathB", bufs=4)
                gp = gpB.__enter__()
                for b in range(PB):
                    L = Ls[b]
                    c0b = int(offs[b])
                    Gt = gp.tile([128, Ls[0] * R1], BF16, tag="g1")
                    Gv = Gt[:, 0:L * R1].rearrange("p (l w) -> p l w", w=R1)
                    for s0 in range(0, L, LCAP):
                        Lc = min(LCAP, L - s0)
                        c0 = c0b + s0
                        nc.gpsimd.dma_gather(
                            out_ap=Gt[:, s0 * R1:(s0 + Lc) * R1].rearrange(
                                "p (l w) -> p l w", w=R1),
                            in_ap=T1full[:],
                            idxs_ap=sidx_sb[:, c0 * 8:(c0 + Lc) * 8],
                            num_idxs=128 * Lc, num_idxs_reg=128 * Lc,
                            elem_size=R1, queue_num=qi % 4)
                        qi += 1
                    if debug_outs and b == 0:
                        nc.sync.dma_start(
                            out=dbg_g[:, 0:L * R1], in_=Gt[:, 0:L * R1])
                    t_t = wp.tile([128, L], F32, tag="tpre")
                    nc.vector.tensor_scalar(
                        out=t_t[:], in0=Gv[:, :, HID],
                        scalar1=sd1[:, b:b + 1], scalar2=None, op0=ALU.add)
                    u_t = wp.tile([128, L], F32, tag="upre")
                    nc.vector.tensor_scalar(
                        out=u_t[:], in0=t_t[:], scalar1=NEG_SLOPE,
                        scalar2=None, op0=ALU.mult)
                    wl_t = wp.tile([128, L], F32, tag="wl1")
                    nc.vector.tensor_tensor(
                        out=wl_t[:], in0=t_t[:], in1=u_t[:], op=ALU.max)
                    wex = wp.tile([128, L], F32, tag="we1")
                    den = wp.tile([128, 1], F32, tag="den")
                    nc.scalar.activation(
                        wex[:], wl_t[:], AF.Exp, accum_out=den[:])
                    wexb = wp.tile([128, L], BF16, tag="web")
                    nc.vector.tensor_copy(wexb[:], wex[:])
                    wt = wtp.tile([128, Ls[0] * HID], BF16, tag="wt")
                    wtv = wt[:, 0:L * HID]
                    nc.vector.tensor_tensor(
                        out=wtv, in0=Gv[:, :, 0:HID],
                        in1=wexb[:, :, None].to_broadcast([128, L, HID]),
                        op=ALU.mult)
                    o_t = wp.tile([128, HID], F32, tag="o1")
                    nc.vector.tensor_reduce(
                        out=o_t[:],
                        in_=wtv.rearrange("p (l f) -> p f l", f=HID),
                        axis=mybir.AxisListType.X, op=ALU.add)
                    # ---- block epilogue: normalize, relu, layer-2 matmul
                    rec = wp.tile([128, 1], F32, tag="rec")
                    nc.vector.reciprocal(rec[:], den[:])
                    ob = wp.tile([128, HID], F32, tag="ob")
                    nc.vector.scalar_tensor_tensor(
                        out=ob[:], in0=o_t[:], scalar=rec[:], in1=b1r_sb[:],
                        op0=ALU.mult, op1=ALU.add)
                    r1 = wp.tile([128, HID], F32, tag="r1")
                    nc.scalar.activation(r1[:], ob[:], AF.Relu)
                    if debug_outs:
                        nc.sync.dma_start(
                            out=dbg_r1[b * 128:(b + 1) * 128, :], in_=r1[:])
                        if b == 0:
                            nc.sync.dma_start(out=dbg_t1[:], in_=T1full[:])
                    pT = psB.tile([128, HID], F32, tag="tr")
                    nc.tensor.transpose(pT[:], r1[:], identity=ident[:])
                    r1T = wp.tile([128, HID], BF16, tag="r1T")
                    nc.vector.tensor_copy(r1T[:], pT[:])
                    ph2 = psB.tile([128, H2W], F32, tag="tr2")
                    nc.tensor.matmul(ph2[:], lhsT=r1T[:], rhs=W2_sb[:],
                                     start=True, stop=True)
                    nc.vector.tensor_copy(
                        T2sb[:, b * R2:b * R2 + HID2 + 1],
                        ph2[:, 0:HID2 + 1])
                    nc.vector.tensor_copy(
                        sd2[:, b:b + 1], ph2[:, HID2 + 1:HID2 + 2])
                    nc.sync.dma_start(
                        out=T2shard[b * 128:(b + 1) * 128, :],
                        in_=T2sb[:, b * R2:(b + 1) * R2])

                gpB.__exit__(None, None, None)
                nc.sync.dma_start(out=T2full[TR - 1:TR, :], in_=sent2_d[:])
                nc.gpsimd.collective_compute(
                    "AllGather", ALU.bypass, replica_groups=groups,
                    ins=[T2shard[:]], outs=[T2full[0:TR - 1, :]])

                # ------------- phase C: GAT layer 2 + pooling -------------
                if phases < 3:
                    out_sb = wp.tile([G, OUT], F32, tag="outsb")
                    nc.vector.memset(out_sb[:], 0.0)
                    nc.sync.dma_start(out=out_d[:], in_=out_sb[:])
                    continue
                pool_ps = psP.tile([G, HID2], F32, tag="pool")
                qi = 0
                gpC = tc.tile_pool(name="gathC", bufs=4)
                gp = gpC.__enter__()
                for b in range(PB):
                    L = Ls[b]
                    c0b = int(offs[b])
                    Gt = gp.tile([128, Ls[0] * R2], BF16, tag="g2")
                    Gv = Gt[:, 0:L * R2].rearrange("p (l w) -> p l w", w=R2)
                    for s0 in range(0, L, LCAP):
                        Lc = min(LCAP, L - s0)
                        c0 = c0b + s0
                        nc.gpsimd.dma_gather(
                            out_ap=Gt[:, s0 * R2:(s0 + Lc) * R2].rearrange(
                                "p (l w) -> p l w", w=R2),
                            in_ap=T2full[:],
                            idxs_ap=sidx_sb[:, c0 * 8:(c0 + Lc) * 8],
                            num_idxs=128 * Lc, num_idxs_reg=128 * Lc,
                            elem_size=R2, queue_num=qi % 4)
                        qi += 1
                    t_t = wp.tile([128, L], F32, tag="tpre")
                    nc.vector.tensor_scalar(
                        out=t_t[:], in0=Gv[:, :, HID2],
                        scalar1=sd2[:, b:b + 1], scalar2=None, op0=ALU.add)
                    u_t = wp.tile([128, L], F32, tag="upre")
                    nc.vector.tensor_scalar(
                        out=u_t[:], in0=t_t[:], scalar1=NEG_SLOPE,
                        scalar2=None, op0=ALU.mult)
                    wl_t = wp.tile([128, L], F32, tag="wl1")
                    nc.vector.tensor_tensor(
                        out=wl_t[:], in0=t_t[:], in1=u_t[:], op=ALU.max)
                    wex = wp.tile([128, L], F32, tag="we1")
                    den = wp.tile([128, 1], F32, tag="den")
                    nc.scalar.activation(
                        wex[:], wl_t[:], AF.Exp, accum_out=den[:])
                    wexb = wp.tile([128, L], BF16, tag="web")
                    nc.vector.tensor_copy(wexb[:], wex[:])
                    wt = wtp.tile([128, Ls[0] * HID2], BF16, tag="wt2")
                    wtv = wt[:, 0:L * HID2]
                    nc.vector.tensor_tensor(
                        out=wtv, in0=Gv[:, :, 0:HID2],
                        in1=wexb[:, :, None].to_broadcast([128, L, HID2]),
                        op=ALU.mult)
                    o_t = wp.tile([128, HID2], F32, tag="o2")
                    nc.vector.tensor_reduce(
                        out=o_t[:],
                        in_=wtv.rearrange("p (l f) -> p f l", f=HID2),
                        axis=mybir.AxisListType.X, op=ALU.add)
                    rec = wp.tile([128, 1], F32, tag="rec")
                    nc.vector.reciprocal(rec[:], den[:])
                    ob = wp.tile([128, HID2], F32, tag="ob2")
                    nc.vector.scalar_tensor_tensor(
                        out=ob[:], in0=o_t[:], scalar=rec[:], in1=b2r_sb[:],
                        op0=ALU.mult, op1=ALU.add)
                    r2 = wp.tile([128, HID2], F32, tag="r2")
                    nc.scalar.activation(r2[:], ob[:], AF.Relu)
                    nc.tensor.matmul(
                        pool_ps[:], lhsT=P_sb[:, b * G:(b + 1) * G],
                        rhs=r2[:], start=(b == 0), stop=(b == PB - 1))

                if phases == 4:
                    out_sb = wp.tile([G, OUT], F32, tag="outsb")
                    nc.vector.memset(out_sb[:], 0.0)
                    nc.sync.dma_start(out=out_d[:], in_=out_sb[:])
                    pooled = wp.tile([G, HID2], F32, tag="pool")
                    nc.vector.tensor_copy(pooled[:], pool_ps[:])
                    continue
                gpC.__exit__(None, None, None)
                pooled = wp.tile([G, HID2], F32, tag="pool")
                nc.vector.tensor_copy(pooled[:], pool_ps[:])
                nc.sync.dma_start(out=poolin[:], in_=pooled[:])
                nc.gpsimd.collective_compute(
                    "AllReduce", ALU.add, replica_groups=groups,
                    ins=[poolin[:]], outs=[poolout[:]])
                pooled_r = wp.tile([G, HID2], F32, tag="poolr")
                nc.sync.dma_start(out=pooled_r[:], in_=poolout[:])
                pTf = psB.tile([HID2, G], F32, tag="tr")
                nc.tensor.transpose(pTf[:], pooled_r[:],
                                    identity=ident[:G, :G])
                fin = wp.tile([HID2 + 1, G], F32, tag="fin")
                nc.vector.tensor_copy(fin[:HID2, :], pTf[:])
                nc.vector.memset(fin[HID2:HID2 + 1, :], 1.0)
                out_ps = psP.tile([G, OUT], F32, tag="tro")
                nc.tensor.matmul(out_ps[:], lhsT=fin[:], rhs=Wl_sb[:],
                                 start=True, stop=True)
                out_sb = wp.tile([G, OUT], F32, tag="outsb")
                nc.vector.tensor_copy(out_sb[:], out_ps[:])
                nc.sync.dma_start(out=out_d[:], in_=out_sb[:])

    nc.compile()
    return nc


# ----------------------------------------------------------------------------
# Entry point
# ----------------------------------------------------------------------------

LAST_RESULTS = None


def kernel(**inputs):
    global LAST_RESULTS
    cfg = full_cfg()
    in_maps, meta = preprocess(cfg=cfg, **inputs)
    nc = build_program(cfg, meta)
    res = run_bass_kernel_spmd(nc, in_maps,
                               core_ids=list(range(cfg["NCORES"])))
    LAST_RESULTS = res
    return np.asarray(res.results[0]["out"], np.float32)
